# revision 1
# baseline (speedup 1.0000x reference)
"""Majority-vote (binary bincount+argmax) Trainium2 Bass kernel.

Problem: inputs [31, 2_000_000] int32 with values in {0, 1}. For each batch
column, output argmax of the class histogram = 1 iff sum of the 31 votes
>= 16 (31 is odd, so no ties), else 0. Output: [2_000_000] int32.

Strategy: pure data-parallel across 8 NeuronCores — each core gets a
contiguous 250_000-column slice, viewed on-chip as [125 partitions, 2000].
Per core: 31 x 1 MB contiguous DMA loads (one per voter row), a serial DVE
int32 accumulate chain, one tensor_scalar is_ge(16) compare, one 1 MB store.
Memory-bound: ~31 MB read / core.
"""

import numpy as np

V = 31                  # voters
BATCH = 2_000_000
N_CORES = 8
B = BATCH // N_CORES    # 250_000 batch columns per core
P = 125                 # SBUF partitions used (125 * 2000 = 250_000)
Q = B // P              # 2000 free elements per partition
NCH = 2                 # free-dim chunks per core
F = Q // NCH            # chunk free size
VT_BUFS = 40            # voter-tile slots (40 * F * 4B = 160 KB/partition)
THRESH = (V + 1) // 2   # 16

_cache = {}


def _build_nc():
    import concourse.bacc as bacc
    import concourse.mybir as mybir
    from concourse.mybir import AluOpType
    from concourse.tile import TileContext

    nc = bacc.Bacc("TRN2", target_bir_lowering=False, debug=False)
    x = nc.dram_tensor("x", [V, P, Q], mybir.dt.int32, kind="ExternalInput")
    y = nc.dram_tensor("y", [P, Q], mybir.dt.int32, kind="ExternalOutput")

    with TileContext(nc) as tc:
        with tc.tile_pool(name="vt", bufs=VT_BUFS) as vpool, \
             tc.tile_pool(name="acc", bufs=2) as apool, \
             tc.tile_pool(name="ot", bufs=2) as opool:
            for ch in range(NCH):
                sl = slice(ch * F, (ch + 1) * F)
                acc = apool.tile([P, F], mybir.dt.int32)
                t0 = None
                for v in range(V):
                    t = vpool.tile([P, F], mybir.dt.int32)
                    eng = nc.sync if v % 2 == 0 else nc.scalar
                    eng.dma_start(t[:], x[v, :, sl])
                    if v == 0:
                        t0 = t
                    elif v == 1:
                        nc.vector.tensor_tensor(acc[:], t0[:], t[:], AluOpType.add)
                    else:
                        nc.vector.tensor_tensor(acc[:], acc[:], t[:], AluOpType.add)
                ot = opool.tile([P, F], mybir.dt.int32)
                nc.vector.tensor_scalar(ot[:], acc[:], THRESH, None, AluOpType.is_ge)
                eng = nc.sync if ch % 2 == 0 else nc.scalar
                eng.dma_start(y[:, sl], ot[:])
    nc.compile()
    return nc


def _get_nc():
    if "nc" not in _cache:
        _cache["nc"] = _build_nc()
    return _cache["nc"]


def _run(in_maps, **kwargs):
    from concourse.bass_utils import run_bass_kernel_spmd

    return run_bass_kernel_spmd(
        _get_nc(), in_maps, core_ids=list(range(N_CORES)), **kwargs
    )


def _shard(inputs):
    in_maps = []
    for i in range(N_CORES):
        xi = np.ascontiguousarray(inputs[:, i * B:(i + 1) * B]).reshape(V, P, Q)
        in_maps.append({"x": xi})
    return in_maps


def _gather(results):
    out = np.empty(BATCH, dtype=np.int32)
    for i in range(N_CORES):
        out[i * B:(i + 1) * B] = results[i]["y"].reshape(B)
    return out


def kernel(inputs: np.ndarray) -> np.ndarray:
    inputs = np.asarray(inputs)
    assert inputs.shape == (V, BATCH), inputs.shape
    inputs = inputs.astype(np.int32, copy=False)
    res = _run(_shard(inputs))
    return _gather(res.results)



# revision 3
# speedup vs baseline: 2.2087x; 2.2087x over previous
"""Majority-vote (binary bincount+argmax) Trainium2 Bass kernel.

Problem: inputs [31, 2_000_000] int32, values in {0, 1}. Output per batch
column: argmax of the 2-class histogram = 1 iff sum of the 31 votes >= 16
(31 odd, no ties), else 0. Output: [2_000_000] int32.

Sharding: pure data parallel over 8 NeuronCores. TRN2's descriptor-
generation engines split one DMA's descriptors across SDMA engines only in
EQUAL chunks: engines_used = largest divisor of descriptor count <= 16, so
a 125-partition DMA uses just 5 of 16 engines (~128 GB/s). We therefore
give each core a 128-partition layout: 250_880 = 128 * 1960 columns per
core with slightly overlapping shards (8 * 250_880 > 2_000_000), and the
host pre-transposes each shard to [128, 31, 1960] so every load DMA has
exactly 128 large contiguous descriptors -> all 16 SDMA engines, ~358 GB/s.

Per core: voter-chunked loads ([128, nv*1960] each, nv = 4,4,4,4,4,4,4,3)
double-buffered against a serial DVE int32 accumulate (full-width adds to
amortize DVE per-op overhead), one is_ge(16) threshold, one store.
"""

import numpy as np

V = 31                   # voters
BATCH = 2_000_000
N_CORES = 8
P = 128                  # SBUF partitions
Q = 1960                 # free elems per partition
B = P * Q                # 250_880 columns per core (shards overlap slightly)
CHUNKS = [4, 4, 4, 4, 4, 4, 4, 3]   # voters per load chunk (sum = 31)
THRESH = (V + 1) // 2    # 16

# Core i processes columns [OFF[i], OFF[i] + B); core 7 is pinned to the end.
OFF = [i * B for i in range(N_CORES - 1)] + [BATCH - B]

_cache = {}


def _build_nc():
    import concourse.bacc as bacc
    import concourse.mybir as mybir
    from concourse.mybir import AluOpType
    from concourse.tile import TileContext

    nc = bacc.Bacc("TRN2", target_bir_lowering=False, debug=False)
    # Host-transposed layout: x[p, v, f] = votes of voter v for column p*Q+f.
    x = nc.dram_tensor("x", [P, V * Q], mybir.dt.int32, kind="ExternalInput")
    y = nc.dram_tensor("y", [P, Q], mybir.dt.int32, kind="ExternalOutput")

    with TileContext(nc) as tc:
        with tc.tile_pool(name="vt", bufs=3) as vpool, \
             tc.tile_pool(name="acc", bufs=1) as apool, \
             tc.tile_pool(name="ot", bufs=1) as opool:
            acc = apool.tile([P, Q], mybir.dt.int32)
            v0 = 0
            for ci, nv in enumerate(CHUNKS):
                t = vpool.tile([P, nv * Q], mybir.dt.int32)
                eng = nc.sync if ci % 2 == 0 else nc.scalar
                eng.dma_start(t[:], x[:, v0 * Q:(v0 + nv) * Q])
                j = 0
                if ci == 0:
                    nc.vector.tensor_tensor(
                        acc[:], t[:, 0:Q], t[:, Q:2 * Q], AluOpType.add
                    )
                    j = 2
                while j < nv:
                    nc.vector.tensor_tensor(
                        acc[:], acc[:], t[:, j * Q:(j + 1) * Q], AluOpType.add
                    )
                    j += 1
                v0 += nv
            ot = opool.tile([P, Q], mybir.dt.int32)
            nc.vector.tensor_scalar(ot[:], acc[:], THRESH, None, AluOpType.is_ge)
            nc.sync.dma_start(y[:], ot[:])
    nc.compile()
    return nc


def _get_nc():
    if "nc" not in _cache:
        _cache["nc"] = _build_nc()
    return _cache["nc"]


def _run(in_maps, **kwargs):
    from concourse.bass_utils import run_bass_kernel_spmd

    return run_bass_kernel_spmd(
        _get_nc(), in_maps, core_ids=list(range(N_CORES)), **kwargs
    )


def _shard(inputs):
    in_maps = []
    for i in range(N_CORES):
        sl = inputs[:, OFF[i]:OFF[i] + B]              # [V, B]
        xi = np.ascontiguousarray(
            sl.reshape(V, P, Q).transpose(1, 0, 2)     # -> [P, V, Q]
        ).reshape(P, V * Q)
        in_maps.append({"x": xi})
    return in_maps


def _gather(results):
    out = np.empty(BATCH, dtype=np.int32)
    for i in range(N_CORES):
        out[OFF[i]:OFF[i] + B] = results[i]["y"].reshape(B)
    return out


def kernel(inputs: np.ndarray) -> np.ndarray:
    inputs = np.asarray(inputs)
    assert inputs.shape == (V, BATCH), inputs.shape
    inputs = inputs.astype(np.int32, copy=False)
    res = _run(_shard(inputs))
    return _gather(res.results)
